# revision 12
# baseline (speedup 1.0000x reference)
"""Trainium2 Bass kernel for CollapsePreventionLoss.

reference:
    atoms = coordinates.reshape(B, N, 3)           # B=64, N=1024
    dist  = sqrt(pairwise_dist_sq + 1e-8)
    loss  = sum_{i<j} relu(2.9 - dist)^2 / B

Strategy (8 NeuronCores, data parallel over batch, 8 batches/core):
  dist_sq[i,j] = s_i + s_j - 2 a_i.a_j  as ONE K=18 bf16 matmul per tile:
  every product is exact in fp32 (bf16 hi/lo split of each coordinate,
  3-way bf16 split of the squared norms), so dist_sq is the exact pair
  distance of slightly-perturbed atoms, plus an EPS_GUARD that keeps it
  positive (sqrt(neg) = NaN on the ACT engine).

  Only upper-triangle block-rows are computed: row-block r (128 rows)
  covers columns [128r, 1024).  The 128x128 diagonal blocks are computed
  unmasked and accumulated separately; on the host the strict-upper part
  is recovered by symmetry: upper = (block_sum - diag_sum_estimate) / 2.

  Pipeline per PSUM tile: PE matmul chunks -> ACT sqrt (PSUM->SBUF, bf16)
  then per batch: DVE t = min(d-2.9, 0) (4x mode) and
  DVE scalar_tensor_tensor (d-2.9)*t with accum_out  ( == relu^2 sums).
  Host sums the [128, 16] per-core partials in fp64.
"""

import sys

for _p in ("/opt/trn_rl_repo",):
    if _p not in sys.path:
        sys.path.insert(0, _p)

import numpy as np

import concourse.bacc as bacc
import concourse.tile as tile
from concourse import mybir
from concourse.bass_utils import run_bass_kernel_spmd

B = 64
N = 1024
NCORES = 8
BPC = B // NCORES  # batches per core

MIN_DISTANCE = 2.9
LOSS_WEIGHT = 1.0
EPS_GUARD = 1e-4  # keeps dist_sq positive despite PSUM accumulation rounding
# (host emulation of the PE fp32 accumulation over this dataset bottoms out
# at dist_sq ~ -7.3e-6; 1e-4 gives >10x margin against sqrt(<0) = NaN)

# dist_sq is computed as an exact-in-fp32 bf16 hi/lo product expansion:
#   a ~= ah + al (bf16 pair), each product bf16 x bf16 is exact in fp32.
#   rows 0-2:   s_i  (3-way bf16 split)  x  1
#   rows 3-14:  4 products per coordinate: (-2ah,ah) (-2ah,al) (-2al,ah) (-2al,al)
#   rows 15-17: 1  x  (s_j + eps) (3-way bf16 split)
K_AUG = 18
P = 128
NRB = N // P  # row blocks per batch

# ---------------------------------------------------------------------------
# PSUM tile plan. Each tile is <= 1024 f32 (2 PSUM banks); chunks never cross
# a 512-col bank boundary.  Tile 0 holds the eight 128-wide diagonal blocks;
# the rest hold each row-block's off-diagonal columns [128(r+1), 1024).
# entries: (tile_width, [(row_block, col_start_local, width, col_start_global)])
TILES = [
    (1024, [(r, 128 * r, 128, 128 * r) for r in range(8)]),          # diagonals
    (896, [(0, 0, 512, 128), (0, 512, 384, 640)]),
    (896, [(1, 0, 512, 256), (1, 512, 256, 768), (6, 768, 128, 896)]),
    (896, [(2, 0, 512, 384), (2, 512, 128, 896), (5, 640, 256, 768)]),
    (896, [(3, 0, 512, 512), (4, 512, 384, 640)]),
]
D_W = sum(tw for tw, _ in TILES)  # 4608
D_OFF = np.cumsum([0] + [tw for tw, _ in TILES])[:-1]
DIAG_W = TILES[0][0]  # 1024: diagonal-block region at d[:, 0:DIAG_W]
GP_BATCHES = 5  # how many batches' off-diag square+sum run on GpSimd

_cache = {}


def _build():
    if "nc" in _cache:
        return _cache["nc"]
    f32 = mybir.dt.float32
    bf16 = mybir.dt.bfloat16
    fp16 = mybir.dt.float16

    nc = bacc.Bacc("TRN2", target_bir_lowering=False, debug=False,
                   enable_asserts=False, num_devices=NCORES)
    lhs_d = nc.dram_tensor("lhs", [BPC, K_AUG, N], bf16, kind="ExternalInput").ap()
    rhs_d = nc.dram_tensor("rhs", [BPC, K_AUG, N], bf16, kind="ExternalInput").ap()
    stats_d = nc.dram_tensor("stats", [P, 2 * BPC], f32, kind="ExternalOutput").ap()

    with tile.TileContext(nc) as tc:
        with (
            tc.tile_pool(name="inp", bufs=4) as inp,
            tc.tile_pool(name="dpool", bufs=2) as dpool,
            tc.tile_pool(name="tpool", bufs=2) as tpool,
            tc.tile_pool(name="sqpool", bufs=2) as sqpool,
            tc.tile_pool(name="spool", bufs=1) as spool,
            tc.tile_pool(name="psum", bufs=4, space="PSUM") as psum,
        ):
            stats_sb = spool.tile([P, 2 * BPC], f32, tag="stats")

            for b in range(BPC):
                lhs_sb = inp.tile([K_AUG, N], bf16, tag="lhs_t")
                rhs_sb = inp.tile([K_AUG, N], bf16, tag="rhs_t")
                nc.sync.dma_start(out=lhs_sb, in_=lhs_d[b])
                nc.sync.dma_start(out=rhs_sb, in_=rhs_d[b])

                d_sb = dpool.tile([P, D_W], fp16, tag="d")
                for g, (tile_w, chunks) in enumerate(TILES):
                    pt = psum.tile([P, tile_w], f32, tag="pt")
                    for r, cs, w, jg in chunks:
                        nc.tensor.matmul(
                            pt[:, cs:cs + w],
                            lhs_sb[:, P * r:P * (r + 1)],
                            rhs_sb[:, jg:jg + w],
                            start=True,
                            stop=True,
                        )
                    nc.scalar.activation(
                        out=d_sb[:, int(D_OFF[g]):int(D_OFF[g]) + tile_w],
                        in_=pt[:, :],
                        func=mybir.ActivationFunctionType.Sqrt,
                        bias=0.0,  # EPS_GUARD is already inside dist_sq
                        scale=1.0,
                    )

                # t = min(d - 2.9, 0)   (== -relu(2.9 - d))
                t_sb = tpool.tile([P, D_W], fp16, tag="t")
                nc.vector.tensor_scalar(
                    out=t_sb,
                    in0=d_sb,
                    scalar1=float(MIN_DISTANCE),
                    scalar2=0.0,
                    op0=mybir.AluOpType.subtract,
                    op1=mybir.AluOpType.min,
                )
                # square + sum == relu(2.9-d)^2 summed.  Diagonal blocks and
                # off-diagonal blocks go to separate stats columns (host
                # halves the diagonal-block sum).
                if b < GP_BATCHES:
                    # GpSimd squares; DVE does the cheap 4x accumulate pass
                    sq_sb = sqpool.tile([P, D_W], fp16, tag="sqg")
                    nc.gpsimd.tensor_tensor(
                        out=sq_sb, in0=t_sb, in1=t_sb,
                        op=mybir.AluOpType.mult,
                    )
                    dmy_d = sqpool.tile([P, DIAG_W], fp16, tag="dmy_d")
                    nc.vector.tensor_scalar(
                        out=dmy_d, in0=sq_sb[:, 0:DIAG_W],
                        scalar1=1.0, scalar2=0.0,
                        op0=mybir.AluOpType.mult,
                        op1=mybir.AluOpType.add,
                        accum_out=stats_sb[:, 2 * b:2 * b + 1],
                    )
                    dmy_o = sqpool.tile([P, D_W - DIAG_W], fp16, tag="dmy_o")
                    nc.vector.tensor_scalar(
                        out=dmy_o, in0=sq_sb[:, DIAG_W:D_W],
                        scalar1=1.0, scalar2=0.0,
                        op0=mybir.AluOpType.mult,
                        op1=mybir.AluOpType.add,
                        accum_out=stats_sb[:, 2 * b + 1:2 * b + 2],
                    )
                else:
                    sqd_sb = sqpool.tile([P, DIAG_W], f32, tag="sqd")
                    nc.vector.scalar_tensor_tensor(
                        out=sqd_sb,
                        in0=d_sb[:, 0:DIAG_W],
                        scalar=float(MIN_DISTANCE),
                        in1=t_sb[:, 0:DIAG_W],
                        op0=mybir.AluOpType.subtract,
                        op1=mybir.AluOpType.mult,
                        accum_out=stats_sb[:, 2 * b:2 * b + 1],
                    )
                    sqo_sb = sqpool.tile([P, D_W - DIAG_W], fp16, tag="sqo")
                    nc.vector.scalar_tensor_tensor(
                        out=sqo_sb,
                        in0=d_sb[:, DIAG_W:D_W],
                        scalar=float(MIN_DISTANCE),
                        in1=t_sb[:, DIAG_W:D_W],
                        op0=mybir.AluOpType.subtract,
                        op1=mybir.AluOpType.mult,
                        accum_out=stats_sb[:, 2 * b + 1:2 * b + 2],
                    )

            nc.sync.dma_start(out=stats_d, in_=stats_sb)

    nc.compile()
    _cache["nc"] = nc
    return nc


def _bf16_split(x, n):
    """Split fp64 array into n bf16 terms summing to ~x."""
    import ml_dtypes

    out = []
    rem = x.copy()
    for _ in range(n):
        h = rem.astype(ml_dtypes.bfloat16)
        out.append(h)
        rem = rem - h.astype(np.float64)
    return out


def _prep_inputs(coords):
    """Host-side: build augmented lhs/rhs per core (bf16 hi/lo expansion)."""
    import ml_dtypes

    bf = ml_dtypes.bfloat16
    atoms = coords.reshape(B, N, 3).astype(np.float64)
    at = atoms.transpose(0, 2, 1)  # [B, 3, N]
    ah = at.astype(bf)
    al = (at - ah.astype(np.float64)).astype(bf)
    a_eff = ah.astype(np.float64) + al.astype(np.float64)
    s_eff = (a_eff * a_eff).sum(axis=1)  # [B, N] exact squared norms of ã

    si = _bf16_split(s_eff, 3)
    sj = _bf16_split(s_eff + EPS_GUARD, 3)

    lhs = np.zeros((B, K_AUG, N), bf)
    rhs = np.zeros((B, K_AUG, N), bf)
    lhs[:, 0], lhs[:, 1], lhs[:, 2] = si
    rhs[:, 0:3] = 1.0
    for c in range(3):
        k = 3 + 4 * c
        m2ah = (-2.0 * ah[:, c].astype(np.float64)).astype(bf)
        m2al = (-2.0 * al[:, c].astype(np.float64)).astype(bf)
        lhs[:, k + 0], rhs[:, k + 0] = m2ah, ah[:, c]
        lhs[:, k + 1], rhs[:, k + 1] = m2ah, al[:, c]
        lhs[:, k + 2], rhs[:, k + 2] = m2al, ah[:, c]
        lhs[:, k + 3], rhs[:, k + 3] = m2al, al[:, c]
    lhs[:, 15:18] = 1.0
    rhs[:, 15], rhs[:, 16], rhs[:, 17] = sj

    in_maps = []
    for c in range(NCORES):
        in_maps.append({
            "lhs": np.ascontiguousarray(lhs[c * BPC:(c + 1) * BPC]),
            "rhs": np.ascontiguousarray(rhs[c * BPC:(c + 1) * BPC]),
        })
    return in_maps


def _diag_estimates(in_maps):
    """Exactly emulate the kernel's value for each true-diagonal element
    (i,i): sequential fp32 accumulation of the 18 exact products, sqrt,
    fp16 rounding, then (d - 2.9) * fp16(d - 2.9).  Returns [B] sums."""
    out = np.empty(B)
    for c in range(NCORES):
        lhs = in_maps[c]["lhs"].astype(np.float64)  # [BPC, 18, N]
        rhs = in_maps[c]["rhs"].astype(np.float64)
        prods = lhs * rhs  # products for (i,i)
        acc = np.zeros((BPC, N), np.float32)
        for k in range(K_AUG):
            acc = (acc + prods[:, k].astype(np.float32)).astype(np.float32)
        d = np.sqrt(acc).astype(np.float16).astype(np.float64)
        t = (d - MIN_DISTANCE).astype(np.float32).astype(np.float16)
        t = np.minimum(t, 0.0).astype(np.float64)
        stt = (d - MIN_DISTANCE) * t                      # DVE path, fp32 out
        ttg = (t * t).astype(np.float32).astype(np.float16).astype(np.float64)
        for b in range(BPC):
            v = ttg[b] if b < GP_BATCHES else stt[b]
            out[c * BPC + b] = v.sum()
    return out


def _run(coordinates, trace=False, **trace_kwargs):
    coords = np.asarray(coordinates, dtype=np.float32)
    assert coords.shape == (B, 3 * N), coords.shape
    nc = _build()
    in_maps = _prep_inputs(coords)
    res = run_bass_kernel_spmd(nc, in_maps, core_ids=list(range(NCORES)),
                               trace=trace, **trace_kwargs)
    diag_est = _diag_estimates(in_maps)
    total = 0.0
    for c in range(NCORES):
        st = res.results[c]["stats"].astype(np.float64)
        for b in range(BPC):
            s_diag = st[:, 2 * b].sum()
            s_off = st[:, 2 * b + 1].sum()
            total += s_off + 0.5 * (s_diag - diag_est[c * BPC + b])
    loss = np.float32(LOSS_WEIGHT * total / B)
    return loss, res


def kernel(coordinates):
    loss, _ = _run(coordinates)
    return np.asarray(loss, dtype=np.float32)


# revision 13
# speedup vs baseline: 1.2700x; 1.2700x over previous
"""Trainium2 Bass kernel for CollapsePreventionLoss.

reference:
    atoms = coordinates.reshape(B, N, 3)           # B=64, N=1024
    dist  = sqrt(pairwise_dist_sq + 1e-8)
    loss  = sum_{i<j} relu(2.9 - dist)^2 / B

Strategy (8 NeuronCores, data parallel over batch, 8 batches/core):
  dist_sq[i,j] = s_i + s_j - 2 a_i.a_j  as ONE K=18 bf16 matmul per tile:
  every product is exact in fp32 (bf16 hi/lo split of each coordinate,
  3-way bf16 split of the squared norms), so dist_sq is the exact pair
  distance of slightly-perturbed atoms, plus an EPS_GUARD that keeps it
  positive (sqrt(neg) = NaN on the ACT engine).

  Only upper-triangle block-rows are computed: row-block r (128 rows)
  covers columns [128r, 1024).  The 128x128 diagonal blocks are computed
  unmasked and accumulated separately; on the host the strict-upper part
  is recovered by symmetry: upper = (block_sum - diag_sum_estimate) / 2.

  Pipeline per PSUM tile: PE matmul chunks -> ACT sqrt (PSUM->SBUF, bf16)
  then per batch: DVE t = min(d-2.9, 0) (4x mode) and
  DVE scalar_tensor_tensor (d-2.9)*t with accum_out  ( == relu^2 sums).
  Host sums the [128, 16] per-core partials in fp64.
"""

import sys

for _p in ("/opt/trn_rl_repo",):
    if _p not in sys.path:
        sys.path.insert(0, _p)

import numpy as np

import concourse.bacc as bacc
import concourse.tile as tile
from concourse import mybir
from concourse.bass_utils import run_bass_kernel_spmd

B = 64
N = 1024
NCORES = 8
BPC = B // NCORES  # batches per core

MIN_DISTANCE = 2.9
LOSS_WEIGHT = 1.0
EPS_GUARD = 1e-4  # keeps dist_sq positive despite PSUM accumulation rounding
# (host emulation of the PE fp32 accumulation over this dataset bottoms out
# at dist_sq ~ -7.3e-6; 1e-4 gives >10x margin against sqrt(<0) = NaN)

# dist_sq is computed as an exact-in-fp32 bf16 hi/lo product expansion:
#   a ~= ah + al (bf16 pair), each product bf16 x bf16 is exact in fp32.
#   rows 0-2:   s_i  (3-way bf16 split)  x  1
#   rows 3-14:  4 products per coordinate: (-2ah,ah) (-2ah,al) (-2al,ah) (-2al,al)
#   rows 15-17: 1  x  (s_j + eps) (3-way bf16 split)
K_AUG = 18
P = 128
NRB = N // P  # row blocks per batch

# ---------------------------------------------------------------------------
# PSUM tile plan. Each tile is <= 1024 f32 (2 PSUM banks); chunks never cross
# a 512-col bank boundary.  Tile 0 holds the eight 128-wide diagonal blocks;
# the rest hold each row-block's off-diagonal columns [128(r+1), 1024).
# entries: (tile_width, [(row_block, col_start_local, width, col_start_global)])
TILES = [
    (1024, [(r, 128 * r, 128, 128 * r) for r in range(8)]),          # diagonals
    (896, [(0, 0, 512, 128), (0, 512, 384, 640)]),
    (896, [(1, 0, 512, 256), (1, 512, 256, 768), (6, 768, 128, 896)]),
    (896, [(2, 0, 512, 384), (2, 512, 128, 896), (5, 640, 256, 768)]),
    (896, [(3, 0, 512, 512), (4, 512, 384, 640)]),
]
D_W = sum(tw for tw, _ in TILES)  # 4608
D_OFF = np.cumsum([0] + [tw for tw, _ in TILES])[:-1]
DIAG_W = TILES[0][0]  # 1024: diagonal-block region at d[:, 0:DIAG_W]
GP_BATCHES = 0  # GpSimd tt offload measured slower; keep all on DVE

_cache = {}


def _build():
    if "nc" in _cache:
        return _cache["nc"]
    f32 = mybir.dt.float32
    bf16 = mybir.dt.bfloat16
    fp16 = mybir.dt.float16

    nc = bacc.Bacc("TRN2", target_bir_lowering=False, debug=False,
                   enable_asserts=False, num_devices=NCORES)
    lhs_d = nc.dram_tensor("lhs", [BPC, K_AUG, N], bf16, kind="ExternalInput").ap()
    rhs_d = nc.dram_tensor("rhs", [BPC, K_AUG, N], bf16, kind="ExternalInput").ap()
    stats_d = nc.dram_tensor("stats", [P, 2 * BPC], f32, kind="ExternalOutput").ap()

    with tile.TileContext(nc) as tc:
        with (
            tc.tile_pool(name="inp", bufs=4) as inp,
            tc.tile_pool(name="dpool", bufs=2) as dpool,
            tc.tile_pool(name="tpool", bufs=2) as tpool,
            tc.tile_pool(name="sqpool", bufs=2) as sqpool,
            tc.tile_pool(name="spool", bufs=1) as spool,
            tc.tile_pool(name="psum", bufs=4, space="PSUM") as psum,
        ):
            stats_sb = spool.tile([P, 2 * BPC], f32, tag="stats")

            for b in range(BPC):
                lhs_sb = inp.tile([K_AUG, N], bf16, tag="lhs_t")
                rhs_sb = inp.tile([K_AUG, N], bf16, tag="rhs_t")
                nc.sync.dma_start(out=lhs_sb, in_=lhs_d[b])
                nc.sync.dma_start(out=rhs_sb, in_=rhs_d[b])

                d_sb = dpool.tile([P, D_W], bf16, tag="d")
                for g, (tile_w, chunks) in enumerate(TILES):
                    pt = psum.tile([P, tile_w], f32, tag="pt")
                    for r, cs, w, jg in chunks:
                        nc.tensor.matmul(
                            pt[:, cs:cs + w],
                            lhs_sb[:, P * r:P * (r + 1)],
                            rhs_sb[:, jg:jg + w],
                            start=True,
                            stop=True,
                        )
                    nc.scalar.activation(
                        out=d_sb[:, int(D_OFF[g]):int(D_OFF[g]) + tile_w],
                        in_=pt[:, :],
                        func=mybir.ActivationFunctionType.Sqrt,
                        bias=0.0,  # EPS_GUARD is already inside dist_sq
                        scale=1.0,
                    )

                # t = min(d - 2.9, 0)   (== -relu(2.9 - d))
                t_sb = tpool.tile([P, D_W], fp16, tag="t")
                nc.vector.tensor_scalar(
                    out=t_sb,
                    in0=d_sb,
                    scalar1=float(MIN_DISTANCE),
                    scalar2=0.0,
                    op0=mybir.AluOpType.subtract,
                    op1=mybir.AluOpType.min,
                )
                # square + sum == relu(2.9-d)^2 summed.  Diagonal blocks and
                # off-diagonal blocks go to separate stats columns (host
                # halves the diagonal-block sum).
                if b < GP_BATCHES:
                    # GpSimd squares; DVE does the cheap 4x accumulate pass
                    sq_sb = sqpool.tile([P, D_W], fp16, tag="sqg")
                    nc.gpsimd.tensor_tensor(
                        out=sq_sb, in0=t_sb, in1=t_sb,
                        op=mybir.AluOpType.mult,
                    )
                    dmy_d = sqpool.tile([P, DIAG_W], fp16, tag="dmy_d")
                    nc.vector.tensor_scalar(
                        out=dmy_d, in0=sq_sb[:, 0:DIAG_W],
                        scalar1=1.0, scalar2=0.0,
                        op0=mybir.AluOpType.mult,
                        op1=mybir.AluOpType.add,
                        accum_out=stats_sb[:, 2 * b:2 * b + 1],
                    )
                    dmy_o = sqpool.tile([P, D_W - DIAG_W], fp16, tag="dmy_o")
                    nc.vector.tensor_scalar(
                        out=dmy_o, in0=sq_sb[:, DIAG_W:D_W],
                        scalar1=1.0, scalar2=0.0,
                        op0=mybir.AluOpType.mult,
                        op1=mybir.AluOpType.add,
                        accum_out=stats_sb[:, 2 * b + 1:2 * b + 2],
                    )
                else:
                    sqd_sb = sqpool.tile([P, DIAG_W], f32, tag="sqd")
                    nc.vector.scalar_tensor_tensor(
                        out=sqd_sb,
                        in0=d_sb[:, 0:DIAG_W],
                        scalar=float(MIN_DISTANCE),
                        in1=t_sb[:, 0:DIAG_W],
                        op0=mybir.AluOpType.subtract,
                        op1=mybir.AluOpType.mult,
                        accum_out=stats_sb[:, 2 * b:2 * b + 1],
                    )
                    sqo_sb = sqpool.tile([P, D_W - DIAG_W], fp16, tag="sqo")
                    nc.vector.scalar_tensor_tensor(
                        out=sqo_sb,
                        in0=d_sb[:, DIAG_W:D_W],
                        scalar=float(MIN_DISTANCE),
                        in1=t_sb[:, DIAG_W:D_W],
                        op0=mybir.AluOpType.subtract,
                        op1=mybir.AluOpType.mult,
                        accum_out=stats_sb[:, 2 * b + 1:2 * b + 2],
                    )

            nc.sync.dma_start(out=stats_d, in_=stats_sb)

    nc.compile()
    _cache["nc"] = nc
    return nc


def _bf16_split(x, n):
    """Split fp64 array into n bf16 terms summing to ~x."""
    import ml_dtypes

    out = []
    rem = x.copy()
    for _ in range(n):
        h = rem.astype(ml_dtypes.bfloat16)
        out.append(h)
        rem = rem - h.astype(np.float64)
    return out


def _prep_inputs(coords):
    """Host-side: build augmented lhs/rhs per core (bf16 hi/lo expansion)."""
    import ml_dtypes

    bf = ml_dtypes.bfloat16
    atoms = coords.reshape(B, N, 3).astype(np.float64)
    at = atoms.transpose(0, 2, 1)  # [B, 3, N]
    ah = at.astype(bf)
    al = (at - ah.astype(np.float64)).astype(bf)
    a_eff = ah.astype(np.float64) + al.astype(np.float64)
    s_eff = (a_eff * a_eff).sum(axis=1)  # [B, N] exact squared norms of ã

    si = _bf16_split(s_eff, 3)
    sj = _bf16_split(s_eff + EPS_GUARD, 3)

    lhs = np.zeros((B, K_AUG, N), bf)
    rhs = np.zeros((B, K_AUG, N), bf)
    lhs[:, 0], lhs[:, 1], lhs[:, 2] = si
    rhs[:, 0:3] = 1.0
    for c in range(3):
        k = 3 + 4 * c
        m2ah = (-2.0 * ah[:, c].astype(np.float64)).astype(bf)
        m2al = (-2.0 * al[:, c].astype(np.float64)).astype(bf)
        lhs[:, k + 0], rhs[:, k + 0] = m2ah, ah[:, c]
        lhs[:, k + 1], rhs[:, k + 1] = m2ah, al[:, c]
        lhs[:, k + 2], rhs[:, k + 2] = m2al, ah[:, c]
        lhs[:, k + 3], rhs[:, k + 3] = m2al, al[:, c]
    lhs[:, 15:18] = 1.0
    rhs[:, 15], rhs[:, 16], rhs[:, 17] = sj

    in_maps = []
    for c in range(NCORES):
        in_maps.append({
            "lhs": np.ascontiguousarray(lhs[c * BPC:(c + 1) * BPC]),
            "rhs": np.ascontiguousarray(rhs[c * BPC:(c + 1) * BPC]),
        })
    return in_maps


def _diag_estimates(in_maps):
    """Exactly emulate the kernel's value for each true-diagonal element
    (i,i): sequential fp32 accumulation of the 18 exact products, sqrt,
    fp16 rounding, then (d - 2.9) * fp16(d - 2.9).  Returns [B] sums."""
    out = np.empty(B)
    for c in range(NCORES):
        lhs = in_maps[c]["lhs"].astype(np.float64)  # [BPC, 18, N]
        rhs = in_maps[c]["rhs"].astype(np.float64)
        prods = lhs * rhs  # products for (i,i)
        acc = np.zeros((BPC, N), np.float32)
        for k in range(K_AUG):
            acc = (acc + prods[:, k].astype(np.float32)).astype(np.float32)
        import ml_dtypes
        d = np.sqrt(acc).astype(ml_dtypes.bfloat16).astype(np.float64)
        t = (d - MIN_DISTANCE).astype(np.float32).astype(np.float16)
        t = np.minimum(t, 0.0).astype(np.float64)
        stt = (d - MIN_DISTANCE) * t                      # DVE path, fp32 out
        ttg = (t * t).astype(np.float32).astype(np.float16).astype(np.float64)
        for b in range(BPC):
            v = ttg[b] if b < GP_BATCHES else stt[b]
            out[c * BPC + b] = v.sum()
    return out


def _run(coordinates, trace=False, **trace_kwargs):
    coords = np.asarray(coordinates, dtype=np.float32)
    assert coords.shape == (B, 3 * N), coords.shape
    nc = _build()
    in_maps = _prep_inputs(coords)
    res = run_bass_kernel_spmd(nc, in_maps, core_ids=list(range(NCORES)),
                               trace=trace, **trace_kwargs)
    diag_est = _diag_estimates(in_maps)
    total = 0.0
    for c in range(NCORES):
        st = res.results[c]["stats"].astype(np.float64)
        for b in range(BPC):
            s_diag = st[:, 2 * b].sum()
            s_off = st[:, 2 * b + 1].sum()
            total += s_off + 0.5 * (s_diag - diag_est[c * BPC + b])
    loss = np.float32(LOSS_WEIGHT * total / B)
    return loss, res


def kernel(coordinates):
    loss, _ = _run(coordinates)
    return np.asarray(loss, dtype=np.float32)


# revision 15
# speedup vs baseline: 1.2780x; 1.0063x over previous
"""Trainium2 Bass kernel for CollapsePreventionLoss.

reference:
    atoms = coordinates.reshape(B, N, 3)           # B=64, N=1024
    dist  = sqrt(pairwise_dist_sq + 1e-8)
    loss  = sum_{i<j} relu(2.9 - dist)^2 / B

Strategy (8 NeuronCores, data parallel over batch, 8 batches/core):
  dist_sq[i,j] = s_i + s_j - 2 a_i.a_j  as ONE K=18 bf16 matmul per tile:
  every product is exact in fp32 (bf16 hi/lo split of each coordinate,
  3-way bf16 split of the squared norms), so dist_sq is the exact pair
  distance of slightly-perturbed atoms, plus an EPS_GUARD that keeps it
  positive (sqrt(neg) = NaN on the ACT engine).

  Only upper-triangle block-rows are computed: row-block r (128 rows)
  covers columns [128r, 1024).  The 128x128 diagonal blocks are computed
  unmasked and accumulated separately; on the host the strict-upper part
  is recovered by symmetry: upper = (block_sum - diag_sum_estimate) / 2.

  Pipeline per PSUM tile: PE matmul chunks -> ACT sqrt (PSUM->SBUF, bf16)
  then per batch: DVE t = min(d-2.9, 0) (4x mode) and
  DVE scalar_tensor_tensor (d-2.9)*t with accum_out  ( == relu^2 sums).
  Host sums the [128, 16] per-core partials in fp64.
"""

import sys

for _p in ("/opt/trn_rl_repo",):
    if _p not in sys.path:
        sys.path.insert(0, _p)

import numpy as np

import concourse.bacc as bacc
import concourse.tile as tile
from concourse import mybir
from concourse.bass_utils import run_bass_kernel_spmd

B = 64
N = 1024
NCORES = 8
BPC = B // NCORES  # batches per core

MIN_DISTANCE = 2.9
LOSS_WEIGHT = 1.0
EPS_GUARD = 1e-4  # keeps dist_sq positive despite PSUM accumulation rounding
# (host emulation of the PE fp32 accumulation over this dataset bottoms out
# at dist_sq ~ -7.3e-6; 1e-4 gives >10x margin against sqrt(<0) = NaN)

# dist_sq is computed as an exact-in-fp32 bf16 hi/lo product expansion:
#   a ~= ah + al (bf16 pair), each product bf16 x bf16 is exact in fp32.
#   rows 0-2:   s_i  (3-way bf16 split)  x  1
#   rows 3-14:  4 products per coordinate: (-2ah,ah) (-2ah,al) (-2al,ah) (-2al,al)
#   rows 15-17: 1  x  (s_j + eps) (3-way bf16 split)
K_AUG = 18
P = 128
NRB = N // P  # row blocks per batch

# ---------------------------------------------------------------------------
# PSUM tile plan. Each tile is <= 1024 f32 (2 PSUM banks); chunks never cross
# a 512-col bank boundary.  Tile 0 holds the eight 128-wide diagonal blocks;
# the rest hold each row-block's off-diagonal columns [128(r+1), 1024).
# entries: (tile_width, [(row_block, col_start_local, width, col_start_global)])
TILES = [
    (1024, [(r, 128 * r, 128, 128 * r) for r in range(8)]),          # diagonals
    (896, [(0, 0, 512, 128), (0, 512, 384, 640)]),
    (896, [(1, 0, 512, 256), (1, 512, 256, 768), (6, 768, 128, 896)]),
    (896, [(2, 0, 512, 384), (2, 512, 128, 896), (5, 640, 256, 768)]),
    (896, [(3, 0, 512, 512), (4, 512, 384, 640)]),
]
D_W = sum(tw for tw, _ in TILES)  # 4608
D_OFF = np.cumsum([0] + [tw for tw, _ in TILES])[:-1]
DIAG_W = TILES[0][0]  # 1024: diagonal-block region at d[:, 0:DIAG_W]
GP_BATCHES = 0  # GpSimd tt offload measured slower; keep all on DVE

_cache = {}


def _build():
    if "nc" in _cache:
        return _cache["nc"]
    f32 = mybir.dt.float32
    bf16 = mybir.dt.bfloat16
    fp16 = mybir.dt.float16

    nc = bacc.Bacc("TRN2", target_bir_lowering=False, debug=False,
                   enable_asserts=False, num_devices=NCORES)
    lhs_d = nc.dram_tensor("lhs", [BPC, K_AUG, N], bf16, kind="ExternalInput").ap()
    rhs_d = nc.dram_tensor("rhs", [BPC, K_AUG, N], bf16, kind="ExternalInput").ap()
    stats_d = nc.dram_tensor("stats", [P, 2 * BPC], f32, kind="ExternalOutput").ap()

    with tile.TileContext(nc) as tc:
        with (
            tc.tile_pool(name="inp", bufs=6) as inp,
            tc.tile_pool(name="dpool", bufs=3) as dpool,
            tc.tile_pool(name="tpool", bufs=3) as tpool,
            tc.tile_pool(name="sqpool", bufs=2) as sqpool,
            tc.tile_pool(name="spool", bufs=1) as spool,
            tc.tile_pool(name="psum", bufs=4, space="PSUM") as psum,
        ):
            stats_sb = spool.tile([P, 2 * BPC], f32, tag="stats")

            for b in range(BPC):
                lhs_sb = inp.tile([K_AUG, N], bf16, tag="lhs_t")
                rhs_sb = inp.tile([K_AUG, N], bf16, tag="rhs_t")
                nc.sync.dma_start(out=lhs_sb, in_=lhs_d[b])
                nc.gpsimd.dma_start(out=rhs_sb, in_=rhs_d[b])

                d_sb = dpool.tile([P, D_W], bf16, tag="d")
                for g, (tile_w, chunks) in enumerate(TILES):
                    pt = psum.tile([P, tile_w], f32, tag="pt")
                    for r, cs, w, jg in chunks:
                        nc.tensor.matmul(
                            pt[:, cs:cs + w],
                            lhs_sb[:, P * r:P * (r + 1)],
                            rhs_sb[:, jg:jg + w],
                            start=True,
                            stop=True,
                        )
                    nc.scalar.activation(
                        out=d_sb[:, int(D_OFF[g]):int(D_OFF[g]) + tile_w],
                        in_=pt[:, :],
                        func=mybir.ActivationFunctionType.Sqrt,
                        bias=0.0,  # EPS_GUARD is already inside dist_sq
                        scale=1.0,
                    )

                # t = min(d - 2.9, 0)   (== -relu(2.9 - d))
                t_sb = tpool.tile([P, D_W], fp16, tag="t")
                nc.vector.tensor_scalar(
                    out=t_sb,
                    in0=d_sb,
                    scalar1=float(MIN_DISTANCE),
                    scalar2=0.0,
                    op0=mybir.AluOpType.subtract,
                    op1=mybir.AluOpType.min,
                )
                # square + sum == relu(2.9-d)^2 summed.  Diagonal blocks and
                # off-diagonal blocks go to separate stats columns (host
                # halves the diagonal-block sum).
                if b < GP_BATCHES:
                    # GpSimd squares; DVE does the cheap 4x accumulate pass
                    sq_sb = sqpool.tile([P, D_W], fp16, tag="sqg")
                    nc.gpsimd.tensor_tensor(
                        out=sq_sb, in0=t_sb, in1=t_sb,
                        op=mybir.AluOpType.mult,
                    )
                    dmy_d = sqpool.tile([P, DIAG_W], fp16, tag="dmy_d")
                    nc.vector.tensor_scalar(
                        out=dmy_d, in0=sq_sb[:, 0:DIAG_W],
                        scalar1=1.0, scalar2=0.0,
                        op0=mybir.AluOpType.mult,
                        op1=mybir.AluOpType.add,
                        accum_out=stats_sb[:, 2 * b:2 * b + 1],
                    )
                    dmy_o = sqpool.tile([P, D_W - DIAG_W], fp16, tag="dmy_o")
                    nc.vector.tensor_scalar(
                        out=dmy_o, in0=sq_sb[:, DIAG_W:D_W],
                        scalar1=1.0, scalar2=0.0,
                        op0=mybir.AluOpType.mult,
                        op1=mybir.AluOpType.add,
                        accum_out=stats_sb[:, 2 * b + 1:2 * b + 2],
                    )
                else:
                    sqd_sb = sqpool.tile([P, DIAG_W], f32, tag="sqd")
                    nc.vector.scalar_tensor_tensor(
                        out=sqd_sb,
                        in0=d_sb[:, 0:DIAG_W],
                        scalar=float(MIN_DISTANCE),
                        in1=t_sb[:, 0:DIAG_W],
                        op0=mybir.AluOpType.subtract,
                        op1=mybir.AluOpType.mult,
                        accum_out=stats_sb[:, 2 * b:2 * b + 1],
                    )
                    sqo_sb = sqpool.tile([P, D_W - DIAG_W], fp16, tag="sqo")
                    nc.vector.scalar_tensor_tensor(
                        out=sqo_sb,
                        in0=d_sb[:, DIAG_W:D_W],
                        scalar=float(MIN_DISTANCE),
                        in1=t_sb[:, DIAG_W:D_W],
                        op0=mybir.AluOpType.subtract,
                        op1=mybir.AluOpType.mult,
                        accum_out=stats_sb[:, 2 * b + 1:2 * b + 2],
                    )

            nc.sync.dma_start(out=stats_d, in_=stats_sb)

    nc.compile()
    _cache["nc"] = nc
    return nc


def _bf16_split(x, n):
    """Split fp64 array into n bf16 terms summing to ~x."""
    import ml_dtypes

    out = []
    rem = x.copy()
    for _ in range(n):
        h = rem.astype(ml_dtypes.bfloat16)
        out.append(h)
        rem = rem - h.astype(np.float64)
    return out


def _prep_inputs(coords):
    """Host-side: build augmented lhs/rhs per core (bf16 hi/lo expansion)."""
    import ml_dtypes

    bf = ml_dtypes.bfloat16
    atoms = coords.reshape(B, N, 3).astype(np.float64)
    at = atoms.transpose(0, 2, 1)  # [B, 3, N]
    ah = at.astype(bf)
    al = (at - ah.astype(np.float64)).astype(bf)
    a_eff = ah.astype(np.float64) + al.astype(np.float64)
    s_eff = (a_eff * a_eff).sum(axis=1)  # [B, N] exact squared norms of ã

    si = _bf16_split(s_eff, 3)
    sj = _bf16_split(s_eff + EPS_GUARD, 3)

    lhs = np.zeros((B, K_AUG, N), bf)
    rhs = np.zeros((B, K_AUG, N), bf)
    lhs[:, 0], lhs[:, 1], lhs[:, 2] = si
    rhs[:, 0:3] = 1.0
    for c in range(3):
        k = 3 + 4 * c
        m2ah = (-2.0 * ah[:, c].astype(np.float64)).astype(bf)
        m2al = (-2.0 * al[:, c].astype(np.float64)).astype(bf)
        lhs[:, k + 0], rhs[:, k + 0] = m2ah, ah[:, c]
        lhs[:, k + 1], rhs[:, k + 1] = m2ah, al[:, c]
        lhs[:, k + 2], rhs[:, k + 2] = m2al, ah[:, c]
        lhs[:, k + 3], rhs[:, k + 3] = m2al, al[:, c]
    lhs[:, 15:18] = 1.0
    rhs[:, 15], rhs[:, 16], rhs[:, 17] = sj

    in_maps = []
    for c in range(NCORES):
        in_maps.append({
            "lhs": np.ascontiguousarray(lhs[c * BPC:(c + 1) * BPC]),
            "rhs": np.ascontiguousarray(rhs[c * BPC:(c + 1) * BPC]),
        })
    return in_maps


def _diag_estimates(in_maps):
    """Exactly emulate the kernel's value for each true-diagonal element
    (i,i): sequential fp32 accumulation of the 18 exact products, sqrt,
    fp16 rounding, then (d - 2.9) * fp16(d - 2.9).  Returns [B] sums."""
    out = np.empty(B)
    for c in range(NCORES):
        lhs = in_maps[c]["lhs"].astype(np.float64)  # [BPC, 18, N]
        rhs = in_maps[c]["rhs"].astype(np.float64)
        prods = lhs * rhs  # products for (i,i)
        acc = np.zeros((BPC, N), np.float32)
        for k in range(K_AUG):
            acc = (acc + prods[:, k].astype(np.float32)).astype(np.float32)
        import ml_dtypes
        d = np.sqrt(acc).astype(ml_dtypes.bfloat16).astype(np.float64)
        t = (d - MIN_DISTANCE).astype(np.float32).astype(np.float16)
        t = np.minimum(t, 0.0).astype(np.float64)
        stt = (d - MIN_DISTANCE) * t                      # DVE path, fp32 out
        ttg = (t * t).astype(np.float32).astype(np.float16).astype(np.float64)
        for b in range(BPC):
            v = ttg[b] if b < GP_BATCHES else stt[b]
            out[c * BPC + b] = v.sum()
    return out


def _run(coordinates, trace=False, **trace_kwargs):
    coords = np.asarray(coordinates, dtype=np.float32)
    assert coords.shape == (B, 3 * N), coords.shape
    nc = _build()
    in_maps = _prep_inputs(coords)
    res = run_bass_kernel_spmd(nc, in_maps, core_ids=list(range(NCORES)),
                               trace=trace, **trace_kwargs)
    diag_est = _diag_estimates(in_maps)
    total = 0.0
    for c in range(NCORES):
        st = res.results[c]["stats"].astype(np.float64)
        for b in range(BPC):
            s_diag = st[:, 2 * b].sum()
            s_off = st[:, 2 * b + 1].sum()
            total += s_off + 0.5 * (s_diag - diag_est[c * BPC + b])
    loss = np.float32(LOSS_WEIGHT * total / B)
    return loss, res


def kernel(coordinates):
    loss, _ = _run(coordinates)
    return np.asarray(loss, dtype=np.float32)


# revision 16
# speedup vs baseline: 1.3623x; 1.0660x over previous
"""Trainium2 Bass kernel for CollapsePreventionLoss.

reference:
    atoms = coordinates.reshape(B, N, 3)           # B=64, N=1024
    dist  = sqrt(pairwise_dist_sq + 1e-8)
    loss  = sum_{i<j} relu(2.9 - dist)^2 / B

Strategy (8 NeuronCores, data parallel over batch, 8 batches/core):
  dist_sq[i,j] = s_i + s_j - 2 a_i.a_j  as ONE K=18 bf16 matmul per tile:
  every product is exact in fp32 (bf16 hi/lo split of each coordinate,
  3-way bf16 split of the squared norms), so dist_sq is the exact pair
  distance of slightly-perturbed atoms, plus an EPS_GUARD that keeps it
  positive (sqrt(neg) = NaN on the ACT engine).

  Only upper-triangle block-rows are computed: row-block r (128 rows)
  covers columns [128r, 1024).  The 128x128 diagonal blocks are computed
  unmasked and accumulated separately; on the host the strict-upper part
  is recovered by symmetry: upper = (block_sum - diag_sum_estimate) / 2.

  Pipeline per PSUM tile: PE matmul chunks -> ACT sqrt (PSUM->SBUF, bf16)
  then per batch: DVE t = min(d-2.9, 0) (4x mode) and
  DVE scalar_tensor_tensor (d-2.9)*t with accum_out  ( == relu^2 sums).
  Host sums the [128, 16] per-core partials in fp64.
"""

import sys

for _p in ("/opt/trn_rl_repo",):
    if _p not in sys.path:
        sys.path.insert(0, _p)

import numpy as np

import concourse.bacc as bacc
import concourse.tile as tile
from concourse import mybir
from concourse.bass_utils import run_bass_kernel_spmd

B = 64
N = 1024
NCORES = 8
BPC = B // NCORES  # batches per core

MIN_DISTANCE = 2.9
LOSS_WEIGHT = 1.0
EPS_GUARD = 1e-4  # keeps dist_sq positive despite PSUM accumulation rounding
# (host emulation of the PE fp32 accumulation over this dataset bottoms out
# at dist_sq ~ -7.3e-6; 1e-4 gives >10x margin against sqrt(<0) = NaN)

# dist_sq is computed as an exact-in-fp32 bf16 hi/lo product expansion:
#   a ~= ah + al (bf16 pair), each product bf16 x bf16 is exact in fp32.
#   rows 0-2:   s_i  (3-way bf16 split)  x  1
#   rows 3-14:  4 products per coordinate: (-2ah,ah) (-2ah,al) (-2al,ah) (-2al,al)
#   rows 15-17: 1  x  (s_j + eps) (3-way bf16 split)
K_AUG = 18
P = 128
NRB = N // P  # row blocks per batch

# ---------------------------------------------------------------------------
# PSUM tile plan. Each tile is <= 1024 f32 (2 PSUM banks); chunks never cross
# a 512-col bank boundary.  Tile 0 holds the eight 128-wide diagonal blocks;
# the rest hold each row-block's off-diagonal columns [128(r+1), 1024).
# entries: (tile_width, [(row_block, col_start_local, width, col_start_global)])
TILES = [
    (1024, [(r, 128 * r, 128, 128 * r) for r in range(8)]),          # diagonals
    (896, [(0, 0, 512, 128), (0, 512, 384, 640)]),
    (896, [(1, 0, 512, 256), (1, 512, 256, 768), (6, 768, 128, 896)]),
    (896, [(2, 0, 512, 384), (2, 512, 128, 896), (5, 640, 256, 768)]),
    (896, [(3, 0, 512, 512), (4, 512, 384, 640)]),
]
D_W = sum(tw for tw, _ in TILES)  # 4608
D_OFF = np.cumsum([0] + [tw for tw, _ in TILES])[:-1]
DIAG_W = TILES[0][0]  # 1024: diagonal-block region at d[:, 0:DIAG_W]
GP_BATCHES = 0  # GpSimd tt offload measured slower; keep all on DVE

_cache = {}


def _build():
    if "nc" in _cache:
        return _cache["nc"]
    f32 = mybir.dt.float32
    bf16 = mybir.dt.bfloat16
    fp16 = mybir.dt.float16

    nc = bacc.Bacc("TRN2", target_bir_lowering=False, debug=False,
                   enable_asserts=False, num_devices=NCORES)
    lhs_d = nc.dram_tensor("lhs", [BPC, K_AUG, N], bf16, kind="ExternalInput").ap()
    rhs_d = nc.dram_tensor("rhs", [BPC, K_AUG, N], bf16, kind="ExternalInput").ap()
    stats_d = nc.dram_tensor("stats", [P, 2 * BPC], f32, kind="ExternalOutput").ap()

    with tile.TileContext(nc) as tc:
        with (
            tc.tile_pool(name="inp", bufs=6) as inp,
            tc.tile_pool(name="dpool", bufs=3) as dpool,
            tc.tile_pool(name="tpool", bufs=3) as tpool,
            tc.tile_pool(name="sqpool", bufs=2) as sqpool,
            tc.tile_pool(name="spool", bufs=1) as spool,
            tc.tile_pool(name="psum", bufs=4, space="PSUM") as psum,
        ):
            stats_sb = spool.tile([P, 2 * BPC], f32, tag="stats")

            for b in range(BPC):
                lhs_sb = inp.tile([K_AUG, N], bf16, tag="lhs_t")
                rhs_sb = inp.tile([K_AUG, N], bf16, tag="rhs_t")
                nc.sync.dma_start(out=lhs_sb, in_=lhs_d[b])
                nc.gpsimd.dma_start(out=rhs_sb, in_=rhs_d[b])

                d_sb = dpool.tile([P, D_W], bf16, tag="d")
                for g, (tile_w, chunks) in enumerate(TILES):
                    pt = psum.tile([P, tile_w], f32, tag="pt")
                    for r, cs, w, jg in chunks:
                        nc.tensor.matmul(
                            pt[:, cs:cs + w],
                            lhs_sb[:, P * r:P * (r + 1)],
                            rhs_sb[:, jg:jg + w],
                            start=True,
                            stop=True,
                        )
                    nc.scalar.activation(
                        out=d_sb[:, int(D_OFF[g]):int(D_OFF[g]) + tile_w],
                        in_=pt[:, :],
                        func=mybir.ActivationFunctionType.Sqrt,
                        bias=0.0,  # EPS_GUARD is already inside dist_sq
                        scale=1.0,
                    )

                # t = min(d - 2.9, 0)   (== -relu(2.9 - d))
                t_sb = tpool.tile([P, D_W], fp16, tag="t")
                nc.vector.tensor_scalar(
                    out=t_sb,
                    in0=d_sb,
                    scalar1=float(MIN_DISTANCE),
                    scalar2=0.0,
                    op0=mybir.AluOpType.subtract,
                    op1=mybir.AluOpType.min,
                )
                # square + sum == relu(2.9-d)^2 summed.  Diagonal blocks and
                # off-diagonal blocks go to separate stats columns (host
                # halves the diagonal-block sum).
                if b < GP_BATCHES:
                    # GpSimd squares; DVE does the cheap 4x accumulate pass
                    sq_sb = sqpool.tile([P, D_W], fp16, tag="sqg")
                    nc.gpsimd.tensor_tensor(
                        out=sq_sb, in0=t_sb, in1=t_sb,
                        op=mybir.AluOpType.mult,
                    )
                    dmy_d = sqpool.tile([P, DIAG_W], fp16, tag="dmy_d")
                    nc.vector.tensor_scalar(
                        out=dmy_d, in0=sq_sb[:, 0:DIAG_W],
                        scalar1=1.0, scalar2=0.0,
                        op0=mybir.AluOpType.mult,
                        op1=mybir.AluOpType.add,
                        accum_out=stats_sb[:, 2 * b:2 * b + 1],
                    )
                    dmy_o = sqpool.tile([P, D_W - DIAG_W], fp16, tag="dmy_o")
                    nc.vector.tensor_scalar(
                        out=dmy_o, in0=sq_sb[:, DIAG_W:D_W],
                        scalar1=1.0, scalar2=0.0,
                        op0=mybir.AluOpType.mult,
                        op1=mybir.AluOpType.add,
                        accum_out=stats_sb[:, 2 * b + 1:2 * b + 2],
                    )
                else:
                    # diagonal-block square+sum on ACT (Square is in the
                    # same table set as Sqrt); off-diag stays on DVE
                    sqd_sb = sqpool.tile([P, DIAG_W], f32, tag="sqd")
                    nc.scalar.activation(
                        out=sqd_sb,
                        in_=t_sb[:, 0:DIAG_W],
                        func=mybir.ActivationFunctionType.Square,
                        bias=0.0,
                        scale=1.0,
                        accum_out=stats_sb[:, 2 * b:2 * b + 1],
                    )
                    sqo_sb = sqpool.tile([P, D_W - DIAG_W], fp16, tag="sqo")
                    nc.vector.scalar_tensor_tensor(
                        out=sqo_sb,
                        in0=d_sb[:, DIAG_W:D_W],
                        scalar=float(MIN_DISTANCE),
                        in1=t_sb[:, DIAG_W:D_W],
                        op0=mybir.AluOpType.subtract,
                        op1=mybir.AluOpType.mult,
                        accum_out=stats_sb[:, 2 * b + 1:2 * b + 2],
                    )

            nc.sync.dma_start(out=stats_d, in_=stats_sb)

    nc.compile()
    _cache["nc"] = nc
    return nc


def _bf16_split(x, n):
    """Split fp64 array into n bf16 terms summing to ~x."""
    import ml_dtypes

    out = []
    rem = x.copy()
    for _ in range(n):
        h = rem.astype(ml_dtypes.bfloat16)
        out.append(h)
        rem = rem - h.astype(np.float64)
    return out


def _prep_inputs(coords):
    """Host-side: build augmented lhs/rhs per core (bf16 hi/lo expansion)."""
    import ml_dtypes

    bf = ml_dtypes.bfloat16
    atoms = coords.reshape(B, N, 3).astype(np.float64)
    at = atoms.transpose(0, 2, 1)  # [B, 3, N]
    ah = at.astype(bf)
    al = (at - ah.astype(np.float64)).astype(bf)
    a_eff = ah.astype(np.float64) + al.astype(np.float64)
    s_eff = (a_eff * a_eff).sum(axis=1)  # [B, N] exact squared norms of ã

    si = _bf16_split(s_eff, 3)
    sj = _bf16_split(s_eff + EPS_GUARD, 3)

    lhs = np.zeros((B, K_AUG, N), bf)
    rhs = np.zeros((B, K_AUG, N), bf)
    lhs[:, 0], lhs[:, 1], lhs[:, 2] = si
    rhs[:, 0:3] = 1.0
    for c in range(3):
        k = 3 + 4 * c
        m2ah = (-2.0 * ah[:, c].astype(np.float64)).astype(bf)
        m2al = (-2.0 * al[:, c].astype(np.float64)).astype(bf)
        lhs[:, k + 0], rhs[:, k + 0] = m2ah, ah[:, c]
        lhs[:, k + 1], rhs[:, k + 1] = m2ah, al[:, c]
        lhs[:, k + 2], rhs[:, k + 2] = m2al, ah[:, c]
        lhs[:, k + 3], rhs[:, k + 3] = m2al, al[:, c]
    lhs[:, 15:18] = 1.0
    rhs[:, 15], rhs[:, 16], rhs[:, 17] = sj

    in_maps = []
    for c in range(NCORES):
        in_maps.append({
            "lhs": np.ascontiguousarray(lhs[c * BPC:(c + 1) * BPC]),
            "rhs": np.ascontiguousarray(rhs[c * BPC:(c + 1) * BPC]),
        })
    return in_maps


def _diag_estimates(in_maps):
    """Exactly emulate the kernel's value for each true-diagonal element
    (i,i): sequential fp32 accumulation of the 18 exact products, sqrt,
    fp16 rounding, then (d - 2.9) * fp16(d - 2.9).  Returns [B] sums."""
    out = np.empty(B)
    for c in range(NCORES):
        lhs = in_maps[c]["lhs"].astype(np.float64)  # [BPC, 18, N]
        rhs = in_maps[c]["rhs"].astype(np.float64)
        prods = lhs * rhs  # products for (i,i)
        acc = np.zeros((BPC, N), np.float32)
        for k in range(K_AUG):
            acc = (acc + prods[:, k].astype(np.float32)).astype(np.float32)
        import ml_dtypes
        d = np.sqrt(acc).astype(ml_dtypes.bfloat16).astype(np.float64)
        t = (d - MIN_DISTANCE).astype(np.float32).astype(np.float16)
        t = np.minimum(t, 0.0).astype(np.float64)
        sq = (t * t).astype(np.float32).astype(np.float64)  # ACT Square path
        for b in range(BPC):
            out[c * BPC + b] = sq[b].sum()
    return out


def _run(coordinates, trace=False, **trace_kwargs):
    coords = np.asarray(coordinates, dtype=np.float32)
    assert coords.shape == (B, 3 * N), coords.shape
    nc = _build()
    in_maps = _prep_inputs(coords)
    res = run_bass_kernel_spmd(nc, in_maps, core_ids=list(range(NCORES)),
                               trace=trace, **trace_kwargs)
    diag_est = _diag_estimates(in_maps)
    total = 0.0
    for c in range(NCORES):
        st = res.results[c]["stats"].astype(np.float64)
        for b in range(BPC):
            s_diag = st[:, 2 * b].sum()
            s_off = st[:, 2 * b + 1].sum()
            total += s_off + 0.5 * (s_diag - diag_est[c * BPC + b])
    loss = np.float32(LOSS_WEIGHT * total / B)
    return loss, res


def kernel(coordinates):
    loss, _ = _run(coordinates)
    return np.asarray(loss, dtype=np.float32)
